# revision 11
# baseline (speedup 1.0000x reference)
import os
import sys
import time

os.environ.setdefault("JAX_PLATFORMS", "axon")
sys.path.insert(0, "/opt/trn_rl_repo")

import numpy as np

# nn_NewsEncoder: hardcoded problem shapes
VOCAB, D, SEQ, H, HD, ATT = 50000, 300, 30, 20, 20, 200
B = 4096
N_CORES = 8
SHARD = B // N_CORES          # 512 items per core
ROWS = SHARD * SEQ            # 15360 rows per core
RC = 480                      # rows per chunk (16 items)
IPC = RC // SEQ               # items per chunk = 16
NCH = ROWS // RC              # 32 chunks
G = 4                         # items per softmax group
NG = IPC // G                 # groups per chunk = 4
F = H * HD                    # 400 concat features
DK = 3                        # d split for matmul contraction
P = 100                       # partition chunk for ctx/hT layouts
DPAD = 384                    # padded embedding dim (col 300 = bias ones)
KD = (128, 128, 44)           # contraction chunk sizes over d' (o-chunks too)
KOFF = (0, 128, 256)

_nc = None
_f8 = None
_bf16 = None


def _dt():
    from concourse import mybir

    return mybir


def _np_dt(dt):
    from concourse import mybir

    return mybir.dt.np(dt)


def _build():
    """Build the Bass graph once per process."""
    global _nc
    if _nc is not None:
        return _nc
    import concourse.bass as bass
    import concourse.tile as tile
    from concourse import bacc, mybir

    f8 = mybir.dt.float8e4
    bf = mybir.dt.bfloat16
    f32 = mybir.dt.float32
    AF = mybir.ActivationFunctionType
    AX = mybir.AxisListType
    ALU = mybir.AluOpType

    nc = bacc.Bacc(trn_type="TRN2")

    # ---- dram I/O ----
    e8 = nc.dram_tensor("e8", [SHARD, SEQ, DPAD], f8, kind="ExternalInput")
    pe4 = nc.dram_tensor("pe4", [4 * SEQ, DPAD], bf, kind="ExternalInput")
    wq = nc.dram_tensor("wq", [128, DK, H * D], bf, kind="ExternalInput")
    wv = nc.dram_tensor("wv", [P, DK, H, HD], bf, kind="ExternalInput")
    wa = nc.dram_tensor("wa", [P, 4, ATT], bf, kind="ExternalInput")
    wq2 = nc.dram_tensor("wq2", [P, 2, 1], bf, kind="ExternalInput")
    ba2 = nc.dram_tensor("ba2", [P, 2], f32, kind="ExternalInput")
    bv2 = nc.dram_tensor("bv2", [HD, H], f32, kind="ExternalInput")
    bq2 = nc.dram_tensor("bq2", [1, 1], f32, kind="ExternalInput")
    zt = nc.dram_tensor("zt", [P, 4, SHARD], f32, kind="ExternalOutput")

    with tile.TileContext(nc) as tc:
        with (
            tc.tile_pool(name="const", bufs=1) as cpool,
            tc.tile_pool(name="et", bufs=2) as etp,
            tc.tile_pool(name="pp", bufs=3) as ppp,
            tc.tile_pool(name="dsc", bufs=1, space="DRAM") as dscp,
            tc.tile_pool(name="er", bufs=1) as erp,
            tc.tile_pool(name="q", bufs=1) as qp,
            tc.tile_pool(name="cx", bufs=1) as cxp,
            tc.tile_pool(name="sm", bufs=1) as smp,
            tc.tile_pool(name="ht", bufs=2) as htp,
            tc.tile_pool(name="zz", bufs=2) as zzp,
            tc.tile_pool(name="pbig", bufs=4, space="PSUM") as pbig,
            tc.tile_pool(name="psc", bufs=2, space="PSUM") as psc,
            tc.tile_pool(name="pcx", bufs=2, space="PSUM") as pcx,
        ):
            # ---- constants / weights resident in SBUF ----
            wq_sb = cpool.tile([128, DK, H * D], bf)
            nc.sync.dma_start(wq_sb[:], wq[:])
            wv_sb = cpool.tile([P, DK, H, HD], bf)
            nc.sync.dma_start(wv_sb[:], wv[:])
            wa_sb = cpool.tile([P, 4, ATT], bf)
            nc.sync.dma_start(wa_sb[:], wa[:])
            wq2_sb = cpool.tile([P, 2, 1], bf)
            nc.sync.dma_start(wq2_sb[:], wq2[:])
            ba_sb = cpool.tile([P, 2], f32)
            nc.sync.dma_start(ba_sb[:], ba2[:])
            bv_sb = cpool.tile([HD, H], f32)
            nc.sync.dma_start(bv_sb[:], bv2[:])
            bq2_sb = cpool.tile([1, 1], f32)
            nc.sync.dma_start(bq2_sb[:], bq2[:])
            ones_sb = cpool.tile([1, P], f32)
            nc.vector.memset(ones_sb[:], 1.0)
            pe4_sb = cpool.tile([4 * SEQ, DPAD], bf)
            nc.sync.dma_start(pe4_sb[:], pe4[:])
            esc = dscp.tile([ROWS, DPAD], bf)
            e8_flat = e8.rearrange("i s d -> (i s) d")

            for ch in range(NCH):
                r0 = ch * RC
                # ---- pre-pass: e = fp8(emb-part) + pe -> bf16 scratch ----
                for t4 in range(4):
                    rr = r0 + t4 * 4 * SEQ
                    ein = ppp.tile([4 * SEQ, DPAD], f8, tag="ein")
                    nc.sync.dma_start(ein[:], e8_flat[rr : rr + 4 * SEQ, :])
                    ebf = ppp.tile([4 * SEQ, DPAD], bf, tag="ebf")
                    nc.vector.scalar_tensor_tensor(
                        ebf[:], ein[:], 0.0625, pe4_sb[:], ALU.mult, ALU.add
                    )
                    nc.sync.dma_start(esc[rr : rr + 4 * SEQ, :], ebf[:])
                # ---- build eT tile on-device via DMA transpose ----
                et_sb = etp.tile([128, DK, RC], bf)
                for c3 in range(DK):
                    nc.sync.dma_start_transpose(
                        et_sb[:, c3, :],
                        esc[r0 : r0 + RC, c3 * 128 : (c3 + 1) * 128],
                    )

                # ---- Q projection: q[o100, dk, h, rc] (fp8) ----
                q_sb = qp.tile([128, DK, H, RC], bf, tag="q")
                eng = 0
                for h in range(H):
                    for ok in range(DK):
                        ow = KD[ok]
                        pq = pbig.tile([128, RC], mybir.dt.float32, tag="pbig")
                        for dk in range(DK):
                            nc.tensor.matmul(
                                pq[:ow],
                                wq_sb[:, dk, h * D + KOFF[ok] : h * D + KOFF[ok] + ow],
                                et_sb[:, dk, :],
                                start=(dk == 0),
                                stop=(dk == DK - 1),
                            )
                        dst = q_sb[:ow, ok, h, :]
                        if eng == 0:
                            nc.scalar.activation(dst, pq[:ow], AF.Copy)
                        else:
                            nc.vector.tensor_copy(dst, pq[:ow])
                        eng ^= 1

                # ---- attention middle, per group of G items ----
                ctx_sb = cxp.tile([P, DK, H, IPC, SEQ], bf, tag="cx")
                for g in range(NG):
                    b0 = g * G  # item index within chunk
                    er_sb = erp.tile([SEQ, G, D], bf)
                    rbase = (ch * IPC + b0) * SEQ
                    src = esc[rbase : rbase + G * SEQ, 0:D].rearrange(
                        "(i s) d -> s i d", s=SEQ
                    )
                    nc.sync.dma_start(er_sb[:], src)

                    exps = smp.tile([SEQ, G, H, SEQ], bf, tag="exps")
                    for bi in range(G):
                        b = b0 + bi
                        rb = b * SEQ
                        for hh in range(2):
                            ps = psc.tile([SEQ, 512], mybir.dt.float32, tag="psc")
                            for dk in range(DK):
                                nc.tensor.matmul(
                                    ps[:, : 10 * SEQ],
                                    et_sb[: KD[dk], dk, rb : rb + SEQ],
                                    q_sb[: KD[dk], dk, hh * 10 : (hh + 1) * 10, rb : rb + SEQ],
                                    start=(dk == 0),
                                    stop=(dk == DK - 1),
                                )
                            nc.scalar.activation(
                                exps[:, bi, hh * 10 : (hh + 1) * 10, :],
                                ps[:, : 10 * SEQ].rearrange("t (h s) -> t h s", h=10),
                                AF.Exp,
                            )
                    # softmax over s (partition-free axis trickery):
                    # sum over s (innermost of [t, g, h, s] is s? no: layout is
                    # [t, g, h, s_q] where s_q is the query index -> reduce X)
                    ssum = smp.tile([SEQ, G, H], f32, tag="ssum")
                    nc.vector.reduce_sum(ssum[:], exps[:], axis=AX.X)
                    rinv = smp.tile([SEQ, G, H], f32, tag="rinv")
                    nc.vector.reciprocal(rinv[:], ssum[:])
                    nc.vector.tensor_tensor(
                        exps[:],
                        exps[:],
                        rinv[:, :, :, None].to_broadcast((SEQ, G, H, SEQ)),
                        ALU.mult,
                    )
                    # ---- ctxT = e_row.T @ expsr : [d, h, s] per item ----
                    for bi in range(G):
                        b = b0 + bi
                        for mt in range(DK):
                            lhs = er_sb[:, bi, mt * P : (mt + 1) * P]
                            for hh in range(2):
                                pc = pcx.tile([P, 512], mybir.dt.float32, tag="pcx")
                                nc.tensor.matmul(
                                    pc[:, : 10 * SEQ],
                                    lhs,
                                    exps[:, bi, hh * 10 : (hh + 1) * 10, :],
                                    start=True,
                                    stop=True,
                                )
                                dst = ctx_sb[:, mt, hh * 10 : (hh + 1) * 10, b, :]
                                src_ap = pc[:, : 10 * SEQ].rearrange(
                                    "p (h s) -> p h s", h=10
                                )
                                if eng == 0:
                                    nc.scalar.activation(dst, src_ap, AF.Copy)
                                else:
                                    nc.vector.tensor_copy(dst, src_ap)
                                eng ^= 1

                # ---- hvT = WvT_h.T @ ctxT_h + bv : -> hT [(h hd) chunks, rc] ----
                ht_sb = htp.tile([P, 4, RC], bf, tag="ht")
                for h in range(H):
                    ph = pbig.tile([P, RC], mybir.dt.float32, tag="pbig")
                    for dk in range(DK):
                        nc.tensor.matmul(
                            ph[:HD, :],
                            wv_sb[:, dk, h, :],
                            ctx_sb[:, dk, h, :, :],
                            start=(dk == 0),
                            stop=(dk == DK - 1),
                        )
                    poff = (h * HD) % P
                    c = (h * HD) // P
                    hvt = htp.tile([HD, RC], bf, tag="hvt")
                    nc.scalar.activation(
                        hvt[:],
                        ph[:HD, :],
                        AF.Identity,
                        bias=bv_sb[:, h : h + 1],
                    )
                    nc.sync.dma_start(ht_sb[poff : poff + HD, c, :], hvt[:])

                # ---- tail: t = tanh(hT.T @ WaT + ba) ----
                tt_sb = htp.tile([P, 2, RC], bf, tag="tt")
                for mt in range(2):
                    pt = pbig.tile([P, RC], mybir.dt.float32, tag="pbig")
                    for kc in range(4):
                        nc.tensor.matmul(
                            pt[:],
                            wa_sb[:, kc, mt * P : (mt + 1) * P],
                            ht_sb[:, kc, :],
                            start=(kc == 0),
                            stop=(kc == 3),
                        )
                    nc.scalar.activation(
                        tt_sb[:, mt, :], pt[:], AF.Tanh, bias=ba_sb[:, mt : mt + 1]
                    )
                # ---- a = t @ wq2T + bq2 : [1, rc] ----
                pa = pbig.tile([P, RC], mybir.dt.float32, tag="pbig")
                for kc in range(2):
                    nc.tensor.matmul(
                        pa[:1, :],
                        wq2_sb[:, kc, :],
                        tt_sb[:, kc, :],
                        start=(kc == 0),
                        stop=(kc == 1),
                    )
                a_sb = htp.tile([1, RC], f32, tag="a")
                nc.scalar.activation(a_sb[:], pa[:1, :], AF.Identity, bias=bq2_sb[:, :1])
                # broadcast a across partitions via PE (ones.T @ a)
                pb = pbig.tile([P, RC], mybir.dt.float32, tag="pbig")
                nc.tensor.matmul(pb[:], ones_sb[:], a_sb[:], start=True, stop=True)
                # ---- z = sum_s a*h ----
                zr = zzp.tile([P, 4, IPC], f32, tag="zr")
                for c4 in range(4):
                    zm = zzp.tile([P, RC], f32, tag="zm")
                    nc.vector.tensor_tensor(
                        zm[:], ht_sb[:, c4, :], pb[:], ALU.mult
                    )
                    nc.vector.reduce_sum(
                        zr[:, c4, :],
                        zm[:].rearrange("p (i s) -> p i s", s=SEQ),
                        axis=AX.X,
                    )
                nc.sync.dma_start(zt[:, :, ch * IPC : (ch + 1) * IPC], zr[:])

    nc.finalize()
    _nc = nc
    return nc


def _prep_const(Wq, bq, Wv, bv, Wa, ba, wq2, bq2, pe):
    import ml_dtypes

    f8 = ml_dtypes.float8_e4m3fn if hasattr(ml_dtypes, "float8_e4m3fn") else ml_dtypes.float8_e4m3
    bf = ml_dtypes.bfloat16

    WqT = Wq.transpose(2, 0, 1).reshape(D, H * D)  # [d, (h o)]
    wq_np = np.zeros((128, DK, H * D), np.float32)
    for dk in range(DK):
        lo = dk * 128
        hi = min(lo + 128, D)
        wq_np[: hi - lo, dk] = WqT[lo:hi]
        if lo <= D < lo + 128:
            wq_np[D - lo, dk] = bq.reshape(H * D)
    wv_np = np.zeros((P, DK, H, HD), np.float32)
    for dk in range(DK):
        wv_np[:, dk] = Wv.transpose(2, 0, 1)[dk * P : (dk + 1) * P]
    wa_np = np.zeros((P, 4, ATT), np.float32)
    for kc in range(4):
        wa_np[:, kc] = Wa.T[kc * P : (kc + 1) * P]
    wq2_np = np.zeros((P, 2, 1), np.float32)
    wq2_np[:, 0, 0] = wq2[0, :P]
    wq2_np[:, 1, 0] = wq2[0, P:]
    ba_np = np.stack([ba[:P], ba[P:]], axis=1).astype(np.float32)
    bv_np = bv.T.astype(np.float32).copy()  # [hd, h]
    bq2_np = bq2.reshape(1, 1).astype(np.float32)
    pe4_np = np.zeros((4 * SEQ, DPAD), np.float32)
    for i in range(4):
        pe4_np[i * SEQ : (i + 1) * SEQ, :D] = pe
    pe4_np[:, D] = 1.0
    return {
        "pe4": pe4_np.astype(bf),
        "wq": wq_np.astype(bf),
        "wv": wv_np.astype(bf),
        "wa": wa_np.astype(bf),
        "wq2": wq2_np.astype(bf),
        "ba2": ba_np,
        "bv2": bv_np,
        "bq2": bq2_np,
    }


def _prep_core(emb_part):
    """emb_part: [SHARD, SEQ, D] f32 emb-lookup only (no pe) -> fp8 padded."""
    import ml_dtypes

    f8 = (
        ml_dtypes.float8_e4m3fn
        if hasattr(ml_dtypes, "float8_e4m3fn")
        else ml_dtypes.float8_e4m3
    )
    ep = np.zeros((SHARD, SEQ, DPAD), f8)
    ep[:, :, :D] = (emb_part * 16.0).astype(f8)
    return {"e8": ep}


_runner = None
_ran_once = False


def _get_runner():
    """Build the sharded jitted executable once per process."""
    global _runner
    if _runner is not None:
        return _runner
    import jax

    try:
        jax.config.update("jax_compilation_cache_dir", "/tmp/jax_neff_cache")
        jax.config.update("jax_persistent_cache_min_compile_time_secs", 0.0)
        jax.config.update("jax_persistent_cache_min_entry_size_bytes", 0)
    except Exception:
        pass
    import numpy as _np
    from jax.sharding import Mesh, PartitionSpec
    from jax.experimental.shard_map import shard_map
    from concourse import mybir
    from concourse.bass2jax import (
        _bass_exec_p,
        install_neuronx_cc_hook,
        partition_id_tensor,
    )

    nc = _build()
    install_neuronx_cc_hook()
    partition_name = nc.partition_id_tensor.name if nc.partition_id_tensor else None

    in_names, out_names, out_avals, zero_outs = [], [], [], []
    for alloc in nc.m.functions[0].allocations:
        if not isinstance(alloc, mybir.MemoryLocationSet):
            continue
        name = alloc.memorylocations[0].name
        if alloc.kind == "ExternalInput":
            if name != partition_name:
                in_names.append(name)
        elif alloc.kind == "ExternalOutput":
            out_names.append(name)
            out_avals.append(
                jax.core.ShapedArray(tuple(alloc.tensor_shape), mybir.dt.np(alloc.dtype))
            )
            zero_outs.append(
                _np.zeros(tuple(alloc.tensor_shape), mybir.dt.np(alloc.dtype))
            )
    n_params = len(in_names)
    n_outs = len(out_names)
    all_in_names = list(in_names) + list(out_names)
    if partition_name is not None:
        all_in_names.append(partition_name)
    donate = tuple(range(n_params, n_params + n_outs))

    def _body(*args):
        operands = list(args)
        if partition_name is not None:
            operands.append(partition_id_tensor())
        outs = _bass_exec_p.bind(
            *operands,
            out_avals=tuple(out_avals),
            in_names=tuple(all_in_names),
            out_names=tuple(out_names),
            lowering_input_output_aliases=(),
            sim_require_finite=True,
            sim_require_nnan=True,
            nc=nc,
        )
        return tuple(outs)

    devices = jax.devices()[:N_CORES]
    mesh = Mesh(np.asarray(devices), ("core",))
    in_specs = (PartitionSpec("core"),) * (n_params + n_outs)
    out_specs = (PartitionSpec("core"),) * n_outs
    fn = jax.jit(
        shard_map(_body, mesh=mesh, in_specs=in_specs, out_specs=out_specs, check_rep=False),
        donate_argnums=donate,
        keep_unused=True,
    )
    _runner = (fn, in_names, out_names, zero_outs)
    return _runner


def kernel(x, emb, pe, Wq, bq, Wv, bv, Wa, ba, wq2, bq2):
    x = np.asarray(x)
    emb = np.asarray(emb, np.float32)
    pe = np.asarray(pe, np.float32)
    const = _prep_const(
        np.asarray(Wq, np.float32),
        np.asarray(bq, np.float32),
        np.asarray(Wv, np.float32),
        np.asarray(bv, np.float32),
        np.asarray(Wa, np.float32),
        np.asarray(ba, np.float32),
        np.asarray(wq2, np.float32),
        np.asarray(bq2, np.float32),
        pe,
    )
    e_full = emb[x]  # [B, SEQ, D] host-side gather (pe added on device)
    per_core = [dict(const) for _ in range(N_CORES)]
    for c in range(N_CORES):
        per_core[c].update(_prep_core(e_full[c * SHARD : (c + 1) * SHARD]))

    global _ran_once
    if not _ran_once:
        # First call: the documented path (compile + run via run_bass_kernel_spmd).
        from concourse.bass_utils import run_bass_kernel_spmd

        nc = _build()
        res = run_bass_kernel_spmd(nc, per_core, core_ids=list(range(N_CORES)))
        _ran_once = True
        zt_all = np.stack(
            [np.asarray(res.results[c]["zt"]) for c in range(N_CORES)], axis=0
        )
    else:
        # Warm calls: same executable, cached jit (skips per-call retrace).
        fn, in_names, out_names, zero_outs = _get_runner()
        concat_in = [
            np.concatenate([per_core[c][k] for c in range(N_CORES)], axis=0)
            for k in in_names
        ]
        concat_zeros = [
            np.zeros((N_CORES * z.shape[0], *z.shape[1:]), z.dtype) for z in zero_outs
        ]
        out_arrs = fn(*concat_in, *concat_zeros)
        zt_all = np.asarray(out_arrs[out_names.index("zt")]).reshape(
            N_CORES, P, 4, SHARD
        )
    out = np.empty((B, F), np.float32)
    for c in range(N_CORES):
        out[c * SHARD : (c + 1) * SHARD] = (
            zt_all[c].astype(np.float32).transpose(2, 1, 0).reshape(SHARD, F)
        )
    return out
